# revision 22
# baseline (speedup 1.0000x reference)
"""DGCNN forward on 8 trn2 NeuronCores (Bass/Tile), data-parallel over B=16.

Self-contained: hardcodes shapes from the problem spec.
  pos [16, 2048, 3] -> logits [16, 40]

Per core: 2 clouds. Per edge-conv layer (d_in, C):
  nd score matmul via augmented operands  lhsT=[x^T; -1], rhs=[x^T; 0.5*|x|^2]
  -> PSUM holds nd[n,j] = x_n.x_j - 0.5|x_j|^2  (rank-equivalent to -d2/2 per row)
  pack low 11 mantissa bits with column index, chunked max8 top-k (exact set,
  up to 8 per 128-chunk), extract indices by masking the packed bits.
  Neighbor features gathered with gpsimd.ap_gather from v^T [C, N] (channels on
  partitions, clouds stacked when C=64); BN stats via on-the-fly sums + one
  AllReduce per layer; BN+LeakyReLU applied with per-partition scalars; max/min
  branches make max_k commute with the monotone BN+LReLU.
"""
import os
import numpy as np

import concourse.bacc as bacc
import concourse.mybir as mybir
import concourse.tile as tile
import concourse.bass_utils as bass_utils

dt = mybir.dt
AT = mybir.AluOpType
AF = mybir.ActivationFunctionType
AX = mybir.AxisListType

N_CORES = 8
B = 16
BL = 2            # clouds per core
N = 2048
K = 20
NCLS = 40
NEG_BIG = -1e30
MASK_HI = 0xFFFFF800
MASK_LO = 0x7FF
GCH = 128                       # gather chunk: points per ap_gather call
DEBUG = int(os.environ.get("DGCNN_DEBUG", "0"))
STAGES = int(os.environ.get("DGCNN_STAGES", "5"))

_CACHE = {}


def _ceil(a, b):
    return (a + b - 1) // b


def _build():
    nc = bacc.Bacc("TRN2", target_bir_lowering=False, debug=False,
                   enable_asserts=True, num_devices=N_CORES)

    f32 = dt.float32
    inp = {}

    def din(name, shape):
        inp[name] = nc.dram_tensor(name, list(shape), f32, kind="ExternalInput").ap()

    din("posT", [BL, 3, N])
    din("wu1", [3, 64]); din("wv1", [3, 64])
    din("wu2", [64, 64]); din("wv2", [64, 64])
    din("wu3", [64, 128]); din("wv3", [64, 128])
    din("g1", [64, 1]); din("b1", [64, 1])
    din("g2", [64, 1]); din("b2", [64, 1])
    din("g3", [128, 1]); din("b3", [128, 1])
    din("Wf", [256, 1024]); din("gf", [1024]); din("bf", [1024])
    din("W4", [1024, 512]); din("g4", [512]); din("be4", [512])
    din("W5", [512, 256]); din("g5", [256]); din("be5", [256])
    din("W6", [256, NCLS]); din("b6", [NCLS, 1])

    out_ap = nc.dram_tensor("out", [B, NCLS], f32, kind="ExternalOutput").ap()
    dbg = {}
    if DEBUG:
        for l in (1, 2, 3):
            dbg[f"idx{l}"] = nc.dram_tensor(
                f"dbgidx{l}", [BL, N, K], dt.float32, kind="ExternalOutput").ap()
        dbg["x1"] = nc.dram_tensor("dbgx1", [128, N], f32, kind="ExternalOutput").ap()
        dbg["x2"] = nc.dram_tensor("dbgx2", [128, N], f32, kind="ExternalOutput").ap()
        dbg["x3"] = nc.dram_tensor("dbgx3", [BL, 128, N], f32, kind="ExternalOutput").ap()
        dbg["gf"] = nc.dram_tensor("dbggf", [B, 1024], f32, kind="ExternalOutput").ap()
        dbg["st1"] = nc.dram_tensor("dbgst1", [128, 5], f32, kind="ExternalOutput").ap()
        dbg["gidx1"] = nc.dram_tensor("dbggidx1", [128, N * K // 16], f32,
                                      kind="ExternalOutput").ap()
        dbg["u1"] = nc.dram_tensor("dbgu1", [128, N], f32, kind="ExternalOutput").ap()
        dbg["vm1"] = nc.dram_tensor("dbgvm1", [128, N], f32, kind="ExternalOutput").ap()

    with tile.TileContext(nc) as tc:
        _emit(nc, tc, inp, out_ap, dbg)
    nc.compile()
    return nc


def _emit(nc, tc, inp, out_ap, dbg):
    f32, i16, i32, u32 = dt.float32, dt.int16, dt.int32, dt.uint32
    with tc.tile_pool(name="persist", bufs=1) as P, \
         tc.tile_pool(name="gdram", bufs=1, space="DRAM") as GD:
        _emit_body(nc, tc, inp, out_ap, dbg, P, GD)


def _emit_body(nc, tc, inp, out_ap, dbg, P, GD):
    f32, i16, i32, u32 = dt.float32, dt.int16, dt.int32, dt.uint32

    # ---- persistent constants ----
    iota_t = P.tile([128, N], u32)
    nc.gpsimd.iota(iota_t[:], pattern=[[1, N]], base=0, channel_multiplier=0)
    mask_hi = P.tile([128, 1], i32)
    nc.gpsimd.memset(mask_hi[:], MASK_HI - (1 << 32))
    mask_lo = P.tile([128, 1], i32)
    nc.gpsimd.memset(mask_lo[:], MASK_LO)
    halfones = P.tile([64, 1], f32)
    nc.gpsimd.memset(halfones[:], 0.5)
    import concourse.masks as masks
    ident = P.tile([128, 128], f32)
    masks.make_identity(nc, ident[:])
    negones = P.tile([1, N], f32)
    nc.vector.memset(negones[:], -1.0)

    # ---- persistent activations ----
    # xb[l][cl]: [d_in+1, N] rows 0:d_in = x^T, last row = 0.5*|x|^2
    xb2 = [P.tile([66, N], f32, name=f"xb2_{c}") for c in range(BL)]
    xb3 = [P.tile([66, N], f32, name=f"xb3_{c}") for c in range(BL)]
    x3T = [P.tile([128, N], f32, name=f"x3T_{c}") for c in range(BL)]

    layers = [
        dict(l=1, d=3, C=64, xb=None, wu="wu1", wv="wv1", g="g1", b="b1"),
        dict(l=2, d=64, C=64, xb=xb2, wu="wu2", wv="wv2", g="g2", b="b2"),
        dict(l=3, d=64, C=128, xb=xb3, wu="wu3", wv="wv3", g="g3", b="b3"),
    ]

    with tc.tile_pool(name="xb1p", bufs=1) as XP:
        xb1 = [XP.tile([5, N], f32, name=f"xb1_{c}") for c in range(BL)]
        for c in range(BL):
            nc.sync.dma_start(xb1[c][0:3, :], inp["posT"][c])
        layers[0]["xb"] = xb1
        _edge_conv(nc, tc, inp, layers[0], layers, iota_t, mask_hi, mask_lo,
                   halfones, ident, negones, xb2, xb3, x3T, dbg)
    if STAGES <= 1:
        return
    for L in layers[1:]:
        if STAGES <= L["l"] - 1:
            return
        _edge_conv(nc, tc, inp, L, layers, iota_t, mask_hi, mask_lo, halfones,
                   ident, negones, xb2, xb3, x3T, dbg)
    if STAGES <= 3:
        return

    with tc.tile_pool(name="fconv", bufs=1) as FP:
        ago = _final_conv(nc, tc, inp, xb2, xb3, x3T, FP, GD)
        with tc.tile_pool(name="cls", bufs=1) as CP:
            _classifier(nc, tc, inp, ago, out_ap, dbg, CP)


def _sqh_row(nc, tc, pool, xb, d, halfones):
    """Fill xb row d with 0.5*sum_c x^2 along columns (per cloud)."""
    f32 = dt.float32
    sq = pool.tile([64, N], f32, name="sq_scr", tag="scr1")
    nc.vector.tensor_tensor(out=sq[0:d, :], in0=xb[0:d, :], in1=xb[0:d, :],
                            op=AT.mult)
    sqr = pool.tile([1, N], f32, name="sqr", tag="sqr")
    with tc.tile_pool(name="psq", bufs=2, space="PSUM") as psq:
        for ch in range(4):
            pr = psq.tile([1, 512], f32, tag="sqrow")
            nc.tensor.matmul(pr[:], halfones[0:d, :],
                             sq[0:d, ch * 512:(ch + 1) * 512],
                             start=True, stop=True)
            nc.scalar.activation(sqr[:, ch * 512:(ch + 1) * 512], pr[:],
                                 AF.Identity)
    return sqr


def _edge_conv(nc, tc, inp, L, layers, iota_t, mask_hi, mask_lo, halfones,
               ident, negones, xb2, xb3, x3T, dbg):
    f32, i16, i32, u32 = dt.float32, dt.int16, dt.int32, dt.uint32
    l, d, C, xb = L["l"], L["d"], L["C"], L["xb"]
    stacked = C == 64          # both clouds share the 128 partitions
    n_t = N // 128             # 16 point tiles
    gch = GCH if C == 64 else 128
    n_gch = N // gch           # gather chunks per cloud

    with tc.tile_pool(name=f"lay{l}", bufs=1) as LP, \
         tc.tile_pool(name=f"lay{l}_dram", bufs=1, space="DRAM") as LD:

        # ---- weights for u = x@(Wtop-Wbot), v = x@Wbot (transposed out) ----
        wu = LP.tile([d, C], f32, name=f"wu{l}")
        wv = LP.tile([d, C], f32, name=f"wv{l}")
        nc.sync.dma_start(wu[:], inp[L["wu"]])
        nc.sync.dma_start(wv[:], inp[L["wv"]])

        # xb = [x; sqh; -1] (rhs), xz = [x; -1; sqh] (lhsT): nd = -d2/2
        xz = [LP.tile([d + 2, N], f32, name=f"xz{l}_{c}") for c in range(BL)]
        for c in range(BL):
            sqr = _sqh_row(nc, tc, LP, xb[c], d, halfones)
            nc.sync.dma_start(xb[c][d:d + 1, :], sqr[:])
            nc.sync.dma_start(xz[c][d + 1:d + 2, :], sqr[:])
            nc.sync.dma_start(xb[c][d + 1:d + 2, :], negones[:])
            nc.sync.dma_start(xz[c][d:d + 1, :], negones[:])
            nc.sync.dma_start(xz[c][0:d, :], xb[c][0:d, :])

        # ---- u^T, v^T ----
        if stacked:
            uT = [LP.tile([128, N], f32, name=f"uT{l}")]
            vT = [LP.tile([128, N], f32, name=f"vT{l}")]
        else:
            uT = [LP.tile([128, N], f32, name=f"uT{l}_{c}") for c in range(BL)]
            vT = [LP.tile([128, N], f32, name=f"vT{l}_{c}") for c in range(BL)]
        with tc.tile_pool(name=f"puv{l}", bufs=2, space="PSUM") as puv:
            for ch in range(4):
                cs = slice(ch * 512, (ch + 1) * 512)
                for (wt, dst) in ((wu, uT), (wv, vT)):
                    if stacked:
                        pt = puv.tile([128, 512], f32, tag="puv")
                        for c in range(BL):
                            nc.tensor.matmul(pt[64 * c:64 * c + 64, :], wt[:],
                                             xb[c][0:d, cs], start=True, stop=True)
                        nc.scalar.activation(dst[0][:, cs], pt[:], AF.Identity)
                    else:
                        for c in range(BL):
                            pt = puv.tile([128, 512], f32, tag="puv")
                            nc.tensor.matmul(pt[:], wt[:], xb[c][0:d, cs],
                                             start=True, stop=True)
                            nc.scalar.activation(dst[c][:, cs], pt[:], AF.Identity)

        # ---- top-K: nd matmul + pack + chunked max8 + index extract ----
        idxd = [LD.tile([N * K], f32, name=f"idxd{l}_{c}") for c in range(BL)]
        with tc.tile_pool(name=f"ptk{l}", bufs=1) as TP:
            idxf = TP.tile([128, 16, K], f32, name="idxf", bufs=2)
            for c in range(BL):
                for t in range(n_t):
                    _topk_tile(nc, tc, TP, xb[c], xz[c], d, t, iota_t, mask_hi,
                               mask_lo, idxf, l)
                # idxf [p, t, k] -> DRAM flat (t*128+p)*K + k
                nc.gpsimd.dma_start(
                    idxd[c][:].rearrange("(t p k) -> p t k", p=128, k=K),
                    idxf[:])
                if DEBUG:
                    nc.sync.dma_start(
                        dbg[f"idx{l}"][c].rearrange("(t p) k -> p t k", p=128),
                        idxf[:])

        # ---- wrapped gather indices ----
        # gidx[r, m] = flat[m*16+r]: transpose M=[2560,16] via PE in [128,16]
        # row-chunks, then convert f32 -> i16 and replicate partition groups.
        nM = N * K // 16          # 2560 columns
        if stacked:
            gidx = [LP.tile([128, nM], i16, name=f"gidx{l}")]
        else:
            gidx = [LP.tile([128, nM], i16, name=f"gidx{l}_{c}")
                    for c in range(BL)]
        with tc.tile_pool(name=f"wr{l}", bufs=1) as WR, \
             tc.tile_pool(name=f"wrp{l}", bufs=2, space="PSUM") as WRP:
            for c in range(BL):
                mb = WR.tile([128, nM // 8], f32, name="mblk", tag="mblk")
                nc.gpsimd.dma_start(
                    mb[:].rearrange("p (c j) -> p c j", j=16),
                    idxd[c][:].rearrange("(c p j) -> p c j", p=128, j=16))
                gf = WR.tile([16, nM], f32, name="gidxf", tag="gidxf")
                for cc in range(K):
                    pt = WRP.tile([16, 128], f32, tag="wrp")
                    nc.tensor.transpose(pt[:], mb[:, cc * 16:(cc + 1) * 16],
                                        ident[:])
                    nc.scalar.activation(gf[:, cc * 128:(cc + 1) * 128], pt[:],
                                         AF.Identity)
                tgt = gidx[0] if stacked else gidx[c]
                base = 64 * c if stacked else 0
                gi16 = WR.tile([16, nM], i16, name="gi16", tag="gi16")
                nc.vector.tensor_copy(gi16[:], gf[:])
                nc.sync.dma_start(tgt[base:base + 16, :], gi16[:])
                if stacked:
                    nc.sync.dma_start(tgt[base + 16:base + 32, :],
                                      tgt[base:base + 16, :])
                    nc.sync.dma_start(tgt[base + 32:base + 64, :],
                                      tgt[base:base + 32, :])
                else:
                    for sh in (16, 32, 64):
                        nc.sync.dma_start(tgt[sh:2 * sh, :], tgt[0:sh, :])

        if DEBUG and l == 1:
            gdbg = LP.tile([128, N * K // 16], f32, name="gdbg")
            nc.vector.tensor_copy(gdbg[:], gidx[0][:])
            nc.sync.dma_start(dbg["gidx1"], gdbg[:])

        # ---- gather + reduce ----
        nV = 1 if stacked else BL
        vmaxT = [LP.tile([128, N], f32, name=f"vmaxT{l}_{i}") for i in range(nV)]
        vminT = [LP.tile([128, N], f32, name=f"vminT{l}_{i}") for i in range(nV)]
        acc = LP.tile([128, 8], f32, name=f"acc{l}")   # sv, suv, sv2 partials
        nc.vector.memset(acc[:, 0:3], 0.0)
        with tc.tile_pool(name=f"pg{l}", bufs=2) as GP:
            for i in range(nV):
                for gc in range(n_gch):
                    _gather_chunk(nc, tc, GP, vT[i], gidx[i], gc, vmaxT[i],
                                  vminT[i], uT[i], acc, l, gch)

        if DEBUG and l == 1:
            nc.sync.dma_start(dbg["u1"], uT[0][:])
            nc.sync.dma_start(dbg["vm1"], vmaxT[0][:])

        # ---- BN stats + AllReduce + scale/shift ----
        s2, t2 = _bn_stats(nc, tc, LP, LD, inp, L, uT, vmaxT, acc, stacked, dbg)

        # ---- apply BN + LReLU + combine branches; emit next xb / x3T ----
        _apply(nc, tc, LP, L, layers, uT, vmaxT, vminT, s2, t2, stacked,
               xb2, xb3, x3T, halfones, dbg)


def _topk_tile(nc, tc, TP, xbc, xzc, d, t, iota_t, mask_hi, mask_lo, idxf, l):
    """Top-K for 128 points (tile t) of one cloud: writes idxf[:, t, :] (f32)."""
    f32, i16, i32, u32 = dt.float32, dt.int16, dt.int32, dt.uint32
    packed = TP.tile([128, N], i32, name="packed", tag="packed")
    with tc.tile_pool(name="pnd", bufs=2, space="PSUM") as PND:
        for ch in range(4):
            cs = slice(ch * 512, (ch + 1) * 512)
            pnd = PND.tile([128, 512], f32, tag="pnd")
            nc.tensor.matmul(pnd[:], xzc[0:d + 2, t * 128:(t + 1) * 128],
                             xbc[0:d + 2, cs], start=True, stop=True)
            nc.vector.scalar_tensor_tensor(
                out=packed[:, cs], in0=pnd[:].bitcast(i32), scalar=mask_hi[:],
                in1=iota_t[:, cs].bitcast(i32),
                op0=AT.bitwise_and, op1=AT.bitwise_or)
    cand = TP.tile([128, 128], f32, name="cand", tag="cand")
    for ch in range(16):
        nc.vector.max(cand[:, ch * 8:(ch + 1) * 8],
                      packed[:, ch * 128:(ch + 1) * 128].bitcast(f32))
    m24 = TP.tile([128, 24], f32, name="m24", tag="m24")
    nc.vector.max(m24[:, 0:8], cand[:])
    nc.vector.match_replace(cand[:], m24[:, 0:8], cand[:], NEG_BIG)
    nc.vector.max(m24[:, 8:16], cand[:])
    nc.vector.match_replace(cand[:], m24[:, 8:16], cand[:], NEG_BIG)
    nc.vector.max(m24[:, 16:24], cand[:])
    idxw = TP.tile([128, 24], i32, name="idxw", tag="idxw")
    nc.vector.tensor_scalar(out=idxw[:], in0=m24[:].bitcast(i32),
                            scalar1=mask_lo[:], scalar2=None, op0=AT.bitwise_and)
    nc.vector.tensor_copy(idxf[:, t, :], idxw[:, 0:K])


def _gather_chunk(nc, tc, GP, vTi, gidxi, gc, vmaxTi, vminTi, uTi, acc, l, gch):
    """Gather gch points' K neighbors, reduce max/min/sum/sumsq."""
    f32 = dt.float32
    ni = gch * K
    cs = slice(gc * gch, (gc + 1) * gch)
    gath = GP.tile([128, ni], f32, name="gath", tag="gath")
    nc.gpsimd.ap_gather(out_ap=gath[:], in_ap=vTi[:],
                        idxs_ap=gidxi[:, gc * (ni // 16):(gc + 1) * (ni // 16)],
                        channels=128, num_elems=N, d=1, num_idxs=ni)
    view = gath[:].rearrange("p (n k) -> p n k", k=K)
    nc.vector.tensor_reduce(out=vmaxTi[:, cs], in_=view, axis=AX.X, op=AT.max)
    nc.vector.tensor_reduce(out=vminTi[:, cs], in_=view, axis=AX.X, op=AT.min)
    tmp = GP.tile([128, gch], f32, name="gtmp", tag="gtmp")
    tmp1 = GP.tile([128, 1], f32, name="gtmp1", tag="gtmp1")
    # vsum chunk
    nc.vector.tensor_reduce(out=tmp[:], in_=view, axis=AX.X, op=AT.add)
    nc.vector.tensor_reduce(out=tmp1[:], in_=tmp[:], axis=AX.X, op=AT.add)
    nc.vector.tensor_tensor(out=acc[:, 0:1], in0=acc[:, 0:1], in1=tmp1[:], op=AT.add)
    # suv chunk: sum_n u*vsum
    nc.vector.tensor_tensor(out=tmp[:], in0=tmp[:], in1=uTi[:, cs], op=AT.mult)
    nc.vector.tensor_reduce(out=tmp1[:], in_=tmp[:], axis=AX.X, op=AT.add)
    nc.vector.tensor_tensor(out=acc[:, 1:2], in0=acc[:, 1:2], in1=tmp1[:], op=AT.add)
    # sumsq: square gathered in place, reduce all
    nc.vector.tensor_tensor(out=gath[:], in0=gath[:], in1=gath[:], op=AT.mult)
    nc.vector.tensor_reduce(out=tmp1[:], in_=gath[:], axis=AX.X, op=AT.add)
    nc.vector.tensor_tensor(out=acc[:, 2:3], in0=acc[:, 2:3], in1=tmp1[:], op=AT.add)


def _bn_stats(nc, tc, LP, LD, inp, L, uT, vmaxT, acc, stacked, dbg):
    """Per-channel mean/var over all (b,n,k); returns (s2, t2) [128,1] tiles."""
    f32 = dt.float32
    l, C = L["l"], L["C"]
    cnt = float(B * N * K)
    scr = LP.tile([128, N], f32, name=f"bnscr{l}", tag="scr2")
    st = LP.tile([128, 5], f32, name=f"st{l}")
    if stacked:
        nc.vector.tensor_reduce(out=st[:, 3:4], in_=uT[0][:], axis=AX.X, op=AT.add)
        nc.vector.tensor_tensor(out=scr[:], in0=uT[0][:], in1=uT[0][:], op=AT.mult)
        nc.vector.tensor_reduce(out=st[:, 4:5], in_=scr[:], axis=AX.X, op=AT.add)
        nc.vector.tensor_copy(st[:, 0:3], acc[:, 0:3])
    else:
        nc.vector.tensor_reduce(out=st[:, 3:4], in_=uT[0][:], axis=AX.X, op=AT.add)
        nc.vector.tensor_reduce(out=scr[:, 0:1], in_=uT[1][:], axis=AX.X, op=AT.add)
        nc.vector.tensor_tensor(out=st[:, 3:4], in0=st[:, 3:4], in1=scr[:, 0:1],
                                op=AT.add)
        nc.vector.tensor_tensor(out=scr[:], in0=uT[0][:], in1=uT[0][:], op=AT.mult)
        nc.vector.tensor_reduce(out=st[:, 4:5], in_=scr[:], axis=AX.X, op=AT.add)
        nc.vector.tensor_tensor(out=scr[:], in0=uT[1][:], in1=uT[1][:], op=AT.mult)
        nc.vector.tensor_reduce(out=scr[:, 0:1], in_=scr[:], axis=AX.X, op=AT.add)
        nc.vector.tensor_tensor(out=st[:, 4:5], in0=st[:, 4:5], in1=scr[:, 0:1],
                                op=AT.add)
        nc.vector.tensor_copy(st[:, 0:3], acc[:, 0:3])

    ard_in = LD.tile([128, 5], f32, name=f"arin{l}")
    ard_out = LD.tile([128, 5], f32, name=f"arout{l}", addr_space="Shared")
    nc.sync.dma_start(ard_in[:], st[:])
    nc.gpsimd.collective_compute(
        "AllReduce", AT.add, ins=[ard_in[:]], outs=[ard_out[:]],
        replica_groups=[list(range(N_CORES))])
    ar = LP.tile([128, 5], f32, name=f"ar{l}")
    nc.sync.dma_start(ar[:], ard_out[:])
    if DEBUG and l == 1:
        nc.sync.dma_start(dbg["st1"], ar[:])

    fin = LP.tile([128, 8], f32, name=f"fin{l}")
    if stacked:
        # fold upper half (cloud 1) onto lower: S = ar[0:64] + ar[64:128]
        fold = LP.tile([64, 5], f32, name=f"fold{l}")
        nc.sync.dma_start(fold[:], ar[64:128, :])
        nc.vector.tensor_tensor(out=ar[0:64, :], in0=ar[0:64, :], in1=fold[:],
                                op=AT.add)
        rows = slice(0, 64)
    else:
        rows = slice(0, 128)
    # S1 = K*su + sv ; S2 = K*su2 + 2*suv + sv2
    nc.vector.tensor_scalar(out=fin[rows, 0:1], in0=ar[rows, 3:4], scalar1=float(K),
                            scalar2=None, op0=AT.mult)
    nc.vector.tensor_tensor(out=fin[rows, 0:1], in0=fin[rows, 0:1],
                            in1=ar[rows, 0:1], op=AT.add)
    nc.vector.tensor_scalar(out=fin[rows, 1:2], in0=ar[rows, 4:5], scalar1=float(K),
                            scalar2=None, op0=AT.mult)
    nc.vector.tensor_scalar(out=fin[rows, 2:3], in0=ar[rows, 1:2], scalar1=2.0,
                            scalar2=None, op0=AT.mult)
    nc.vector.tensor_tensor(out=fin[rows, 1:2], in0=fin[rows, 1:2],
                            in1=fin[rows, 2:3], op=AT.add)
    nc.vector.tensor_tensor(out=fin[rows, 1:2], in0=fin[rows, 1:2],
                            in1=ar[rows, 2:3], op=AT.add)
    # m = S1/cnt ; E2 = S2/cnt ; var = E2 - m^2
    nc.vector.tensor_scalar(out=fin[rows, 0:1], in0=fin[rows, 0:1],
                            scalar1=1.0 / cnt, scalar2=None, op0=AT.mult)
    nc.vector.tensor_scalar(out=fin[rows, 1:2], in0=fin[rows, 1:2],
                            scalar1=1.0 / cnt, scalar2=None, op0=AT.mult)
    nc.vector.tensor_tensor(out=fin[rows, 2:3], in0=fin[rows, 0:1],
                            in1=fin[rows, 0:1], op=AT.mult)
    nc.vector.tensor_tensor(out=fin[rows, 1:2], in0=fin[rows, 1:2],
                            in1=fin[rows, 2:3], op=AT.subtract)
    # rstd = 1/sqrt(var+eps)
    nc.vector.tensor_scalar(out=fin[rows, 1:2], in0=fin[rows, 1:2], scalar1=1e-5,
                            scalar2=None, op0=AT.add)
    nc.scalar.activation(fin[rows, 3:4], fin[rows, 1:2], AF.Sqrt)
    nc.vector.reciprocal(fin[rows, 4:5], fin[rows, 3:4])

    gb = LP.tile([128, 2], f32, name=f"gb{l}")
    if stacked:
        for c in range(2):
            nc.sync.dma_start(gb[64 * c:64 * c + 64, 0:1], inp[L["g"]])
            nc.sync.dma_start(gb[64 * c:64 * c + 64, 1:2], inp[L["b"]])
    else:
        nc.sync.dma_start(gb[:, 0:1], inp[L["g"]])
        nc.sync.dma_start(gb[:, 1:2], inp[L["b"]])

    s2 = LP.tile([128, 1], f32, name=f"s2_{l}")
    t2 = LP.tile([128, 1], f32, name=f"t2_{l}")
    nc.vector.tensor_tensor(out=s2[rows], in0=gb[rows, 0:1], in1=fin[rows, 4:5],
                            op=AT.mult)
    nc.vector.tensor_tensor(out=t2[rows], in0=fin[rows, 0:1], in1=s2[rows],
                            op=AT.mult)
    nc.vector.tensor_tensor(out=t2[rows], in0=gb[rows, 1:2], in1=t2[rows],
                            op=AT.subtract)
    if stacked:
        nc.sync.dma_start(s2[64:128, :], s2[0:64, :])
        nc.sync.dma_start(t2[64:128, :], t2[0:64, :])
    return s2, t2


def _apply(nc, tc, LP, L, layers, uT, vmaxT, vminT, s2, t2, stacked,
           xb2, xb3, x3T, halfones, dbg):
    """x_out = max(lrelu(s*(u+vmax)+t), lrelu(s*(u+vmin)+t)); route to next xb."""
    f32 = dt.float32
    l = L["l"]
    nV = 1 if stacked else BL
    for i in range(nV):
        z1 = LP.tile([128, N], f32, name="z1", tag="scr1")
        z2 = LP.tile([128, N], f32, name="z2", tag="scr2")
        nc.vector.tensor_tensor(out=z1[:], in0=uT[i][:], in1=vmaxT[i][:], op=AT.add)
        nc.vector.tensor_tensor(out=z2[:], in0=uT[i][:], in1=vminT[i][:], op=AT.add)
        # y = s*z + t on ACT; lrelu + max on DVE
        nc.scalar.activation(z1[:], z1[:], AF.Identity, bias=t2[:], scale=s2[:])
        nc.scalar.activation(z2[:], z2[:], AF.Identity, bias=t2[:], scale=s2[:])
        nc.vector.scalar_tensor_tensor(out=z1[:], in0=z1[:], scalar=0.2, in1=z1[:],
                                       op0=AT.mult, op1=AT.max)
        nc.vector.scalar_tensor_tensor(out=z2[:], in0=z2[:], scalar=0.2, in1=z2[:],
                                       op0=AT.mult, op1=AT.max)
        xout = x3T[i] if l == 3 else z1
        nc.vector.tensor_tensor(out=xout[:], in0=z1[:], in1=z2[:], op=AT.max)
        if l < 3:
            nxt = xb2 if l == 1 else xb3
            # cloud 0 rows 0:64 stay put; cloud 1 rows 64:128 shift via DMA
            nc.sync.dma_start(nxt[0][0:64, :], xout[0:64, :])
            nc.sync.dma_start(nxt[1][0:64, :], xout[64:128, :])
            if DEBUG:
                nc.sync.dma_start(dbg[f"x{l}"], xout[:])
        elif DEBUG:
            nc.sync.dma_start(dbg["x3"][i], xout[:])


def _final_conv(nc, tc, inp, xb2, xb3, x3T, FP, FD):
    """xf^T = Wf^T xc^T per cloud; per-channel max/min over n + BN sums + AR.
    Returns the all-gathered gfeat DRAM tile [16, 1024]."""
    f32 = dt.float32

    wf0 = FP.tile([64, 1024], f32)
    wf1 = FP.tile([64, 1024], f32)
    wf2 = FP.tile([128, 1024], f32)
    nc.sync.dma_start(wf0[:], inp["Wf"][0:64, :])
    nc.sync.dma_start(wf1[:], inp["Wf"][64:128, :])
    nc.sync.dma_start(wf2[:], inp["Wf"][128:256, :])

    ssum = FP.tile([128, 8], f32)
    ssum2 = FP.tile([128, 8], f32)
    smax = FP.tile([128, 16], f32)
    smin = FP.tile([128, 16], f32)
    nc.vector.memset(ssum[:], 0.0)
    nc.vector.memset(ssum2[:], 0.0)
    nc.vector.memset(smax[:], NEG_BIG)
    nc.vector.memset(smin[:], -NEG_BIG)

    f32r = dt.float32r
    with tc.tile_pool(name="pfc", bufs=3, space="PSUM") as PFC, \
         tc.tile_pool(name="fctmp", bufs=2) as FT:
        for c in range(BL):
            for ot in range(8):
                ws = slice(ot * 128, (ot + 1) * 128)
                for nch in range(4):
                    cs = slice(nch * 512, (nch + 1) * 512)
                    pb = PFC.tile([128, 512], f32, tag="pfc")
                    nc.tensor.matmul(pb[:], wf0[:, ws], xb2[c][0:64, cs],
                                     start=True, stop=False)
                    nc.tensor.matmul(pb[:], wf1[:, ws], xb3[c][0:64, cs],
                                     start=False, stop=False)
                    nc.tensor.matmul(pb[:], wf2[:, ws], x3T[c][:, cs],
                                     start=False, stop=True)
                    t1 = FT.tile([128, 1], f32, tag="fct1")
                    sq = FT.tile([128, 512], f32, tag="fcsq")
                    mcol = smax[:, c * 8 + ot:c * 8 + ot + 1]
                    ncol = smin[:, c * 8 + ot:c * 8 + ot + 1]
                    nc.vector.tensor_reduce(out=t1[:], in_=pb[:], axis=AX.X, op=AT.max)
                    nc.vector.tensor_tensor(out=mcol, in0=mcol, in1=t1[:], op=AT.max)
                    nc.vector.tensor_reduce(out=t1[:], in_=pb[:], axis=AX.X, op=AT.min)
                    nc.vector.tensor_tensor(out=ncol, in0=ncol, in1=t1[:], op=AT.min)
                    scol = ssum[:, ot:ot + 1]
                    qcol = ssum2[:, ot:ot + 1]
                    nc.vector.tensor_reduce(out=t1[:], in_=pb[:], axis=AX.X, op=AT.add)
                    nc.vector.tensor_tensor(out=scol, in0=scol, in1=t1[:], op=AT.add)
                    nc.scalar.activation(sq[:], pb[:], AF.Square)
                    nc.vector.tensor_reduce(out=t1[:], in_=sq[:], axis=AX.X, op=AT.add)
                    nc.vector.tensor_tensor(out=qcol, in0=qcol, in1=t1[:], op=AT.add)

    ard_in = FD.tile([128, 16], f32)
    ard_out = FD.tile([128, 16], f32, addr_space="Shared")
    nc.sync.dma_start(ard_in[:, 0:8], ssum[:])
    nc.sync.dma_start(ard_in[:, 8:16], ssum2[:])
    nc.gpsimd.collective_compute(
        "AllReduce", AT.add, ins=[ard_in[:]], outs=[ard_out[:]],
        replica_groups=[list(range(N_CORES))])
    ar = FP.tile([128, 16], f32)
    nc.sync.dma_start(ar[:], ard_out[:])

    cntf = float(B * N)
    m = FP.tile([128, 8], f32)
    var = FP.tile([128, 8], f32)
    nc.vector.tensor_scalar(out=m[:], in0=ar[:, 0:8], scalar1=1.0 / cntf,
                            scalar2=None, op0=AT.mult)
    nc.vector.tensor_scalar(out=var[:], in0=ar[:, 8:16], scalar1=1.0 / cntf,
                            scalar2=None, op0=AT.mult)
    sq2 = FP.tile([128, 8], f32)
    nc.vector.tensor_tensor(out=sq2[:], in0=m[:], in1=m[:], op=AT.mult)
    nc.vector.tensor_tensor(out=var[:], in0=var[:], in1=sq2[:], op=AT.subtract)
    nc.vector.tensor_scalar(out=var[:], in0=var[:], scalar1=1e-5, scalar2=None,
                            op0=AT.add)
    nc.scalar.activation(var[:], var[:], AF.Sqrt)
    rstd = FP.tile([128, 8], f32)
    nc.vector.reciprocal(rstd[:], var[:])
    gfb = FP.tile([128, 16], f32)
    nc.sync.dma_start(gfb[:, 0:8], inp["gf"].rearrange("(a p) -> p a", p=128))
    nc.sync.dma_start(gfb[:, 8:16], inp["bf"].rearrange("(a p) -> p a", p=128))
    sF = FP.tile([128, 8], f32)
    tF = FP.tile([128, 8], f32)
    nc.vector.tensor_tensor(out=sF[:], in0=gfb[:, 0:8], in1=rstd[:], op=AT.mult)
    nc.vector.tensor_tensor(out=tF[:], in0=m[:], in1=sF[:], op=AT.mult)
    nc.vector.tensor_tensor(out=tF[:], in0=gfb[:, 8:16], in1=tF[:], op=AT.subtract)

    # gfeat[cl, o] = max(lrelu(s*smax+t), lrelu(s*smin+t))
    gfe = FP.tile([128, 16], f32)   # cols cl*8+ot
    y1 = FP.tile([128, 1], f32)
    y2 = FP.tile([128, 1], f32)
    for c in range(BL):
        for ot in range(8):
            col = c * 8 + ot
            nc.vector.tensor_scalar(out=y1[:], in0=smax[:, col:col + 1],
                                    scalar1=sF[:, ot:ot + 1], scalar2=tF[:, ot:ot + 1],
                                    op0=AT.mult, op1=AT.add)
            nc.vector.scalar_tensor_tensor(out=y1[:], in0=y1[:], scalar=0.2,
                                           in1=y1[:], op0=AT.mult, op1=AT.max)
            nc.vector.tensor_scalar(out=y2[:], in0=smin[:, col:col + 1],
                                    scalar1=sF[:, ot:ot + 1], scalar2=tF[:, ot:ot + 1],
                                    op0=AT.mult, op1=AT.add)
            nc.vector.scalar_tensor_tensor(out=y2[:], in0=y2[:], scalar=0.2,
                                           in1=y2[:], op0=AT.mult, op1=AT.max)
            nc.vector.tensor_tensor(out=gfe[:, col:col + 1], in0=y1[:], in1=y2[:],
                                    op=AT.max)

    agi = FD.tile([BL, 1024], f32)
    ago = FD.tile([B, 1024], f32, addr_space="Shared")
    nc.sync.dma_start(
        agi[:].rearrange("c (ot p) -> p c ot", p=128), gfe[:])
    nc.gpsimd.collective_compute(
        "AllGather", AT.bypass, ins=[agi[:]], outs=[ago[:]],
        replica_groups=[list(range(N_CORES))])
    return ago


def _classifier(nc, tc, inp, ago, out_ap, dbg, CP):
    """Replicated tiny classifier on the all-gathered gfeat [16, 1024]."""
    f32 = dt.float32

    gT = CP.tile([128, 8 * B], f32)   # [p, (ot, b)]
    for ot in range(8):
        nc.gpsimd.dma_start(
            gT[:, ot * B:(ot + 1) * B],
            ago[:, ot * 128:(ot + 1) * 128].rearrange("b p -> p b"))
    if DEBUG:
        nc.sync.dma_start(dbg["gf"], ago[:])

    w4 = CP.tile([128, 8 * 512], f32)
    nc.sync.dma_start(w4[:].rearrange("p (rc o) -> p rc o", rc=8),
                      inp["W4"].rearrange("(rc p) o -> p rc o", p=128))
    w5 = CP.tile([128, 4 * 256], f32)
    nc.sync.dma_start(w5[:].rearrange("p (rc o) -> p rc o", rc=4),
                      inp["W5"].rearrange("(rc p) o -> p rc o", p=128))
    w6 = CP.tile([128, 2 * NCLS], f32)
    nc.sync.dma_start(w6[:].rearrange("p (rc o) -> p rc o", rc=2),
                      inp["W6"].rearrange("(rc p) o -> p rc o", p=128))

    def bn_block(pre, nblk, gname, bname):
        """pre: psum [128, nblk*B]; returns SBUF tile post BN+LReLU."""
        h = CP.tile([128, nblk * B], f32, name=f"h_{gname}")
        gb = CP.tile([128, 2 * nblk], f32, name=f"gb_{gname}")
        nc.sync.dma_start(gb[:, 0:nblk],
                          inp[gname].rearrange("(a p) -> p a", p=128))
        nc.sync.dma_start(gb[:, nblk:2 * nblk],
                          inp[bname].rearrange("(a p) -> p a", p=128))
        mt = CP.tile([128, 1], f32, name=f"mt_{gname}")
        vt = CP.tile([128, 1], f32, name=f"vt_{gname}")
        sq = CP.tile([128, B], f32, name=f"sqc_{gname}", tag=f"sq_{gname}")
        for j in range(nblk):
            bs = slice(j * B, (j + 1) * B)
            nc.vector.tensor_reduce(out=mt[:], in_=pre[:, bs], axis=AX.X, op=AT.add)
            nc.vector.tensor_scalar(out=mt[:], in0=mt[:], scalar1=1.0 / B,
                                    scalar2=None, op0=AT.mult)
            nc.scalar.activation(sq[:], pre[:, bs], AF.Square)
            nc.vector.tensor_reduce(out=vt[:], in_=sq[:], axis=AX.X, op=AT.add)
            nc.vector.tensor_scalar(out=vt[:], in0=vt[:], scalar1=1.0 / B,
                                    scalar2=None, op0=AT.mult)
            nc.vector.tensor_tensor(out=sq[:, 0:1], in0=mt[:], in1=mt[:], op=AT.mult)
            nc.vector.tensor_tensor(out=vt[:], in0=vt[:], in1=sq[:, 0:1],
                                    op=AT.subtract)
            nc.vector.tensor_scalar(out=vt[:], in0=vt[:], scalar1=1e-5,
                                    scalar2=None, op0=AT.add)
            nc.scalar.activation(vt[:], vt[:], AF.Sqrt)
            nc.vector.reciprocal(vt[:], vt[:])
            nc.vector.tensor_tensor(out=vt[:], in0=vt[:], in1=gb[:, j:j + 1],
                                    op=AT.mult)   # s
            nc.vector.tensor_tensor(out=mt[:], in0=mt[:], in1=vt[:], op=AT.mult)
            nc.vector.tensor_tensor(out=mt[:], in0=gb[:, nblk + j:nblk + j + 1],
                                    in1=mt[:], op=AT.subtract)   # t
            nc.vector.tensor_scalar(out=h[:, bs], in0=pre[:, bs], scalar1=vt[:],
                                    scalar2=mt[:], op0=AT.mult, op1=AT.add)
            nc.vector.scalar_tensor_tensor(out=h[:, bs], in0=h[:, bs], scalar=0.2,
                                           in1=h[:, bs], op0=AT.mult, op1=AT.max)
        return h

    with tc.tile_pool(name="pcls", bufs=2, space="PSUM") as PC:
        p1 = PC.tile([128, 4 * B], f32)
        for o2 in range(4):
            for rc in range(8):
                nc.tensor.matmul(p1[:, o2 * B:(o2 + 1) * B],
                                 w4[:, rc * 512 + o2 * 128:rc * 512 + (o2 + 1) * 128],
                                 gT[:, rc * B:(rc + 1) * B],
                                 start=(rc == 0), stop=(rc == 7))
        h1 = bn_block(p1[:], 4, "g4", "be4")

        p2 = PC.tile([128, 2 * B], f32)
        for o3 in range(2):
            for rc in range(4):
                nc.tensor.matmul(p2[:, o3 * B:(o3 + 1) * B],
                                 w5[:, rc * 256 + o3 * 128:rc * 256 + (o3 + 1) * 128],
                                 h1[:, rc * B:(rc + 1) * B],
                                 start=(rc == 0), stop=(rc == 3))
        h2 = bn_block(p2[:], 2, "g5", "be5")

        p3 = PC.tile([NCLS, B], f32)
        for rc in range(2):
            nc.tensor.matmul(p3[:], w6[:, rc * NCLS:(rc + 1) * NCLS],
                             h2[:, rc * B:(rc + 1) * B],
                             start=(rc == 0), stop=(rc == 1))
        b6 = CP.tile([NCLS, 1], f32)
        nc.sync.dma_start(b6[:], inp["b6"])
        lg = CP.tile([NCLS, B], f32)
        nc.vector.tensor_scalar(out=lg[:], in0=p3[:], scalar1=b6[:], scalar2=None,
                                op0=AT.add)
        nc.sync.dma_start(out_ap.rearrange("b c -> c b"), lg[:])


def _prep_inputs(inputs):
    """Host-side: per-core shards + weight splits (layout-only prep)."""
    pos = np.ascontiguousarray(inputs["pos"], dtype=np.float32)
    W1, W2, W3 = inputs["W1"], inputs["W2"], inputs["W3"]
    common = {
        "wu1": np.ascontiguousarray(W1[0:3] - W1[3:6], np.float32),
        "wv1": np.ascontiguousarray(W1[3:6], np.float32),
        "wu2": np.ascontiguousarray(W2[0:64] - W2[64:128], np.float32),
        "wv2": np.ascontiguousarray(W2[64:128], np.float32),
        "wu3": np.ascontiguousarray(W3[0:64] - W3[64:128], np.float32),
        "wv3": np.ascontiguousarray(W3[64:128], np.float32),
    }
    for k in ("Wf", "gf", "bf", "W4", "g4", "be4", "W5", "g5", "be5", "W6"):
        common[k] = np.ascontiguousarray(inputs[k], np.float32)
    for k in ("g1", "b1", "g2", "b2", "g3", "b3", "b6"):
        common[k] = np.ascontiguousarray(inputs[k], np.float32).reshape(-1, 1)
    in_maps = []
    for i in range(N_CORES):
        m = dict(common)
        m["posT"] = np.ascontiguousarray(
            pos[i * BL:(i + 1) * BL].transpose(0, 2, 1), np.float32)
        in_maps.append(m)
    return in_maps


def kernel(**inputs) -> np.ndarray:
    if "nc" not in _CACHE:
        _CACHE["nc"] = _build()
    nc = _CACHE["nc"]
    in_maps = _prep_inputs(inputs)
    res = bass_utils.run_bass_kernel_spmd(
        nc, in_maps, core_ids=list(range(N_CORES)),
        trace=bool(int(os.environ.get("DGCNN_TRACE", "0"))))
    _CACHE["last_results"] = res
    return np.ascontiguousarray(res.results[0]["out"])


# revision 25
# speedup vs baseline: 1.0393x; 1.0393x over previous
"""DGCNN forward on 8 trn2 NeuronCores (Bass/Tile), data-parallel over B=16.

Self-contained: hardcodes shapes from the problem spec.
  pos [16, 2048, 3] -> logits [16, 40]

Per core: 2 clouds. Per edge-conv layer (d_in, C):
  nd score matmul via augmented operands  lhsT=[x^T; -1], rhs=[x^T; 0.5*|x|^2]
  -> PSUM holds nd[n,j] = x_n.x_j - 0.5|x_j|^2  (rank-equivalent to -d2/2 per row)
  pack low 11 mantissa bits with column index, chunked max8 top-k (exact set,
  up to 8 per 128-chunk), extract indices by masking the packed bits.
  Neighbor features gathered with gpsimd.ap_gather from v^T [C, N] (channels on
  partitions, clouds stacked when C=64); BN stats via on-the-fly sums + one
  AllReduce per layer; BN+LeakyReLU applied with per-partition scalars; max/min
  branches make max_k commute with the monotone BN+LReLU.
"""
import os
import numpy as np

import concourse.bacc as bacc
import concourse.mybir as mybir
import concourse.tile as tile
import concourse.bass_utils as bass_utils

dt = mybir.dt
AT = mybir.AluOpType
AF = mybir.ActivationFunctionType
AX = mybir.AxisListType

N_CORES = 8
B = 16
BL = 2            # clouds per core
N = 2048
K = 20
NCLS = 40
NEG_BIG = -1e30
MASK_HI = 0xFFFFF800
MASK_LO = 0x7FF
GCH = 128                       # gather chunk: points per ap_gather call
DEBUG = int(os.environ.get("DGCNN_DEBUG", "0"))
STAGES = int(os.environ.get("DGCNN_STAGES", "5"))

_CACHE = {}


def _ceil(a, b):
    return (a + b - 1) // b


def _build():
    nc = bacc.Bacc("TRN2", target_bir_lowering=False, debug=False,
                   enable_asserts=True, num_devices=N_CORES)

    f32 = dt.float32
    inp = {}

    def din(name, shape):
        inp[name] = nc.dram_tensor(name, list(shape), f32, kind="ExternalInput").ap()

    din("posT", [BL, 3, N])
    din("wu1", [3, 64]); din("wv1", [3, 64])
    din("wu2", [64, 64]); din("wv2", [64, 64])
    din("wu3", [64, 128]); din("wv3", [64, 128])
    din("g1", [64, 1]); din("b1", [64, 1])
    din("g2", [64, 1]); din("b2", [64, 1])
    din("g3", [128, 1]); din("b3", [128, 1])
    din("Wf", [256, 1024]); din("gf", [1024]); din("bf", [1024])
    din("W4", [1024, 512]); din("g4", [512]); din("be4", [512])
    din("W5", [512, 256]); din("g5", [256]); din("be5", [256])
    din("W6", [256, NCLS]); din("b6", [NCLS, 1])

    out_ap = nc.dram_tensor("out", [B, NCLS], f32, kind="ExternalOutput").ap()
    dbg = {}
    if DEBUG:
        for l in (1, 2, 3):
            dbg[f"idx{l}"] = nc.dram_tensor(
                f"dbgidx{l}", [BL, N, K], dt.float32, kind="ExternalOutput").ap()
        dbg["x1"] = nc.dram_tensor("dbgx1", [128, N], f32, kind="ExternalOutput").ap()
        dbg["x2"] = nc.dram_tensor("dbgx2", [128, N], f32, kind="ExternalOutput").ap()
        dbg["x3"] = nc.dram_tensor("dbgx3", [BL, 128, N], f32, kind="ExternalOutput").ap()
        dbg["gf"] = nc.dram_tensor("dbggf", [B, 1024], f32, kind="ExternalOutput").ap()
        dbg["st1"] = nc.dram_tensor("dbgst1", [128, 5], f32, kind="ExternalOutput").ap()
        dbg["gidx1"] = nc.dram_tensor("dbggidx1", [128, N * K // 16], f32,
                                      kind="ExternalOutput").ap()
        dbg["u1"] = nc.dram_tensor("dbgu1", [128, N], f32, kind="ExternalOutput").ap()
        dbg["vm1"] = nc.dram_tensor("dbgvm1", [128, N], f32, kind="ExternalOutput").ap()

    with tile.TileContext(nc) as tc:
        _emit(nc, tc, inp, out_ap, dbg)
    nc.compile()
    return nc


def _emit(nc, tc, inp, out_ap, dbg):
    f32, i16, i32, u32 = dt.float32, dt.int16, dt.int32, dt.uint32
    with tc.tile_pool(name="persist", bufs=1) as P, \
         tc.tile_pool(name="gdram", bufs=1, space="DRAM") as GD:
        _emit_body(nc, tc, inp, out_ap, dbg, P, GD)


def _emit_body(nc, tc, inp, out_ap, dbg, P, GD):
    f32, i16, i32, u32 = dt.float32, dt.int16, dt.int32, dt.uint32

    # ---- persistent constants ----
    iota_t = P.tile([128, N], u32)
    nc.gpsimd.iota(iota_t[:], pattern=[[1, N]], base=0, channel_multiplier=0)
    mask_hi = P.tile([128, 1], i32)
    nc.gpsimd.memset(mask_hi[:], MASK_HI - (1 << 32))
    mask_lo = P.tile([128, 1], i32)
    nc.gpsimd.memset(mask_lo[:], MASK_LO)
    halfones = P.tile([64, 1], f32)
    nc.gpsimd.memset(halfones[:], 0.5)
    import concourse.masks as masks
    ident = P.tile([128, 128], f32)
    masks.make_identity(nc, ident[:])
    negones = P.tile([1, N], f32)
    nc.vector.memset(negones[:], -1.0)

    # ---- persistent activations ----
    # xb[l][cl]: [d_in+1, N] rows 0:d_in = x^T, last row = 0.5*|x|^2
    xb2 = [P.tile([66, N], f32, name=f"xb2_{c}") for c in range(BL)]
    xb3 = [P.tile([66, N], f32, name=f"xb3_{c}") for c in range(BL)]
    x3T = [P.tile([128, N], f32, name=f"x3T_{c}") for c in range(BL)]

    layers = [
        dict(l=1, d=3, C=64, xb=None, wu="wu1", wv="wv1", g="g1", b="b1"),
        dict(l=2, d=64, C=64, xb=xb2, wu="wu2", wv="wv2", g="g2", b="b2"),
        dict(l=3, d=64, C=128, xb=xb3, wu="wu3", wv="wv3", g="g3", b="b3"),
    ]

    with tc.tile_pool(name="xb1p", bufs=1) as XP:
        xb1 = [XP.tile([5, N], f32, name=f"xb1_{c}") for c in range(BL)]
        for c in range(BL):
            nc.sync.dma_start(xb1[c][0:3, :], inp["posT"][c])
        layers[0]["xb"] = xb1
        _edge_conv(nc, tc, inp, layers[0], layers, iota_t, mask_hi, mask_lo,
                   halfones, ident, negones, xb2, xb3, x3T, dbg)
    if STAGES <= 1:
        return
    for L in layers[1:]:
        if STAGES <= L["l"] - 1:
            return
        _edge_conv(nc, tc, inp, L, layers, iota_t, mask_hi, mask_lo, halfones,
                   ident, negones, xb2, xb3, x3T, dbg)
    if STAGES <= 3:
        return

    with tc.tile_pool(name="fconv", bufs=1) as FP:
        ago = _final_conv(nc, tc, inp, xb2, xb3, x3T, FP, GD)
        with tc.tile_pool(name="cls", bufs=1) as CP:
            _classifier(nc, tc, inp, ago, out_ap, dbg, CP)


def _sqh_row(nc, tc, pool, xb, d, halfones):
    """Fill xb row d with 0.5*sum_c x^2 along columns (per cloud)."""
    f32 = dt.float32
    sq = pool.tile([64, N], f32, name="sq_scr", tag="scr1")
    nc.vector.tensor_tensor(out=sq[0:d, :], in0=xb[0:d, :], in1=xb[0:d, :],
                            op=AT.mult)
    sqr = pool.tile([1, N], f32, name="sqr", tag="sqr", bufs=2)
    with tc.tile_pool(name="psq", bufs=2, space="PSUM") as psq:
        for ch in range(4):
            pr = psq.tile([1, 512], f32, tag="sqrow")
            nc.tensor.matmul(pr[:], halfones[0:d, :],
                             sq[0:d, ch * 512:(ch + 1) * 512],
                             start=True, stop=True)
            nc.scalar.activation(sqr[:, ch * 512:(ch + 1) * 512], pr[:],
                                 AF.Identity)
    return sqr


def _edge_conv(nc, tc, inp, L, layers, iota_t, mask_hi, mask_lo, halfones,
               ident, negones, xb2, xb3, x3T, dbg):
    f32, i16, i32, u32 = dt.float32, dt.int16, dt.int32, dt.uint32
    l, d, C, xb = L["l"], L["d"], L["C"], L["xb"]
    stacked = C == 64          # both clouds share the 128 partitions
    n_t = N // 128             # 16 point tiles
    gch = GCH if C == 64 else 64
    n_gch = N // gch           # gather chunks per cloud

    with tc.tile_pool(name=f"lay{l}", bufs=1) as LP, \
         tc.tile_pool(name=f"lay{l}_dram", bufs=1, space="DRAM") as LD:

        # ---- weights for u = x@(Wtop-Wbot), v = x@Wbot (transposed out) ----
        wu = LP.tile([d, C], f32, name=f"wu{l}")
        wv = LP.tile([d, C], f32, name=f"wv{l}")
        nc.sync.dma_start(wu[:], inp[L["wu"]])
        nc.sync.dma_start(wv[:], inp[L["wv"]])

        # xb = [x; sqh; -1] (rhs), xz = [x; -1; sqh] (lhsT): nd = -d2/2
        xz = [LP.tile([d + 2, N], f32, name=f"xz{l}_{c}") for c in range(BL)]
        for c in range(BL):
            sqr = _sqh_row(nc, tc, LP, xb[c], d, halfones)
            nc.sync.dma_start(xb[c][d:d + 1, :], sqr[:])
            nc.sync.dma_start(xz[c][d + 1:d + 2, :], sqr[:])
            nc.sync.dma_start(xb[c][d + 1:d + 2, :], negones[:])
            nc.sync.dma_start(xz[c][d:d + 1, :], negones[:])
            nc.sync.dma_start(xz[c][0:d, :], xb[c][0:d, :])

        # ---- u^T, v^T ----
        if stacked:
            uT = [LP.tile([128, N], f32, name=f"uT{l}")]
            vT = [LP.tile([128, N], f32, name=f"vT{l}")]
        else:
            uT = [LP.tile([128, N], f32, name=f"uT{l}_{c}") for c in range(BL)]
            vT = [LP.tile([128, N], f32, name=f"vT{l}_{c}") for c in range(BL)]
        with tc.tile_pool(name=f"puv{l}", bufs=2, space="PSUM") as puv:
            for ch in range(4):
                cs = slice(ch * 512, (ch + 1) * 512)
                for (wt, dst) in ((wu, uT), (wv, vT)):
                    if stacked:
                        pt = puv.tile([128, 512], f32, tag="puv")
                        for c in range(BL):
                            nc.tensor.matmul(pt[64 * c:64 * c + 64, :], wt[:],
                                             xb[c][0:d, cs], start=True, stop=True)
                        nc.scalar.activation(dst[0][:, cs], pt[:], AF.Identity)
                    else:
                        for c in range(BL):
                            pt = puv.tile([128, 512], f32, tag="puv")
                            nc.tensor.matmul(pt[:], wt[:], xb[c][0:d, cs],
                                             start=True, stop=True)
                            nc.scalar.activation(dst[c][:, cs], pt[:], AF.Identity)

        # ---- top-K: nd matmul + pack + chunked max8 + index extract ----
        idxd = [LD.tile([N * K], f32, name=f"idxd{l}_{c}") for c in range(BL)]
        with tc.tile_pool(name=f"ptk{l}", bufs=1) as TP, \
             tc.tile_pool(name=f"pnd{l}", bufs=4, space="PSUM") as PND:
            idxf = TP.tile([128, 16, K], f32, name="idxf", bufs=2)
            for c in range(BL):
                for t in range(n_t):
                    _topk_tile(nc, tc, TP, PND, xb[c], xz[c], d, t, iota_t,
                               mask_hi, mask_lo, idxf, l)
                # idxf [p, t, k] -> DRAM flat (t*128+p)*K + k
                nc.gpsimd.dma_start(
                    idxd[c][:].rearrange("(t p k) -> p t k", p=128, k=K),
                    idxf[:])
                if DEBUG:
                    nc.sync.dma_start(
                        dbg[f"idx{l}"][c].rearrange("(t p) k -> p t k", p=128),
                        idxf[:])

        # ---- wrapped gather indices ----
        # gidx[r, m] = flat[m*16+r]: transpose M=[2560,16] via PE in [128,16]
        # row-chunks, then convert f32 -> i16 and replicate partition groups.
        nM = N * K // 16          # 2560 columns
        if stacked:
            gidx = [LP.tile([128, nM], i16, name=f"gidx{l}")]
        else:
            gidx = [LP.tile([128, nM], i16, name=f"gidx{l}_{c}")
                    for c in range(BL)]
        with tc.tile_pool(name=f"wr{l}", bufs=1) as WR, \
             tc.tile_pool(name=f"wrp{l}", bufs=2, space="PSUM") as WRP:
            for c in range(BL):
                mb = WR.tile([128, nM // 8], f32, name="mblk", tag="mblk")
                nc.gpsimd.dma_start(
                    mb[:].rearrange("p (c j) -> p c j", j=16),
                    idxd[c][:].rearrange("(c p j) -> p c j", p=128, j=16))
                gf = WR.tile([16, nM], f32, name="gidxf", tag="gidxf")
                for cc in range(K):
                    pt = WRP.tile([16, 128], f32, tag="wrp")
                    nc.tensor.transpose(pt[:], mb[:, cc * 16:(cc + 1) * 16],
                                        ident[:])
                    nc.scalar.activation(gf[:, cc * 128:(cc + 1) * 128], pt[:],
                                         AF.Identity)
                tgt = gidx[0] if stacked else gidx[c]
                base = 64 * c if stacked else 0
                gi16 = WR.tile([16, nM], i16, name="gi16", tag="gi16")
                nc.vector.tensor_copy(gi16[:], gf[:])
                nc.sync.dma_start(tgt[base:base + 16, :], gi16[:])
                if stacked:
                    nc.sync.dma_start(tgt[base + 16:base + 32, :],
                                      tgt[base:base + 16, :])
                    nc.sync.dma_start(tgt[base + 32:base + 64, :],
                                      tgt[base:base + 32, :])
                else:
                    for sh in (16, 32, 64):
                        nc.sync.dma_start(tgt[sh:2 * sh, :], tgt[0:sh, :])

        if DEBUG and l == 1:
            gdbg = LP.tile([128, N * K // 16], f32, name="gdbg")
            nc.vector.tensor_copy(gdbg[:], gidx[0][:])
            nc.sync.dma_start(dbg["gidx1"], gdbg[:])

        # ---- gather + reduce ----
        nV = 1 if stacked else BL
        vmaxT = [LP.tile([128, N], f32, name=f"vmaxT{l}_{i}") for i in range(nV)]
        vminT = [LP.tile([128, N], f32, name=f"vminT{l}_{i}") for i in range(nV)]
        acc = LP.tile([128, 8], f32, name=f"acc{l}")   # sv, suv, sv2 partials
        nc.vector.memset(acc[:, 0:3], 0.0)
        with tc.tile_pool(name=f"pg{l}", bufs=2) as GP:
            for i in range(nV):
                for gc in range(n_gch):
                    _gather_chunk(nc, tc, GP, vT[i], gidx[i], gc, vmaxT[i],
                                  vminT[i], uT[i], acc, l, gch)

        if DEBUG and l == 1:
            nc.sync.dma_start(dbg["u1"], uT[0][:])
            nc.sync.dma_start(dbg["vm1"], vmaxT[0][:])

        # ---- BN stats + AllReduce + scale/shift ----
        s2, t2 = _bn_stats(nc, tc, LP, LD, inp, L, uT, vmaxT, acc, stacked, dbg)

        # ---- apply BN + LReLU + combine branches; emit next xb / x3T ----
        _apply(nc, tc, LP, L, layers, uT, vmaxT, vminT, s2, t2, stacked,
               xb2, xb3, x3T, halfones, dbg)


def _topk_tile(nc, tc, TP, PND, xbc, xzc, d, t, iota_t, mask_hi, mask_lo,
               idxf, l):
    """Top-K for 128 points (tile t) of one cloud: writes idxf[:, t, :] (f32)."""
    f32, i16, i32, u32 = dt.float32, dt.int16, dt.int32, dt.uint32
    packed = TP.tile([128, N], i32, name="packed", tag="packed", bufs=2)
    for ch in range(4):
        cs = slice(ch * 512, (ch + 1) * 512)
        pnd = PND.tile([128, 512], f32, tag="pnd")
        nc.tensor.matmul(pnd[:], xzc[0:d + 2, t * 128:(t + 1) * 128],
                         xbc[0:d + 2, cs], start=True, stop=True)
        nc.vector.scalar_tensor_tensor(
            out=packed[:, cs], in0=pnd[:].bitcast(i32), scalar=mask_hi[:],
            in1=iota_t[:, cs].bitcast(i32),
            op0=AT.bitwise_and, op1=AT.bitwise_or)
    cand = TP.tile([128, 128], f32, name="cand", tag="cand", bufs=2)
    for ch in range(16):
        nc.vector.max(cand[:, ch * 8:(ch + 1) * 8],
                      packed[:, ch * 128:(ch + 1) * 128].bitcast(f32))
    m24 = TP.tile([128, 24], f32, name="m24", tag="m24", bufs=2)
    nc.vector.max(m24[:, 0:8], cand[:])
    nc.vector.match_replace(cand[:], m24[:, 0:8], cand[:], NEG_BIG)
    nc.vector.max(m24[:, 8:16], cand[:])
    nc.vector.match_replace(cand[:], m24[:, 8:16], cand[:], NEG_BIG)
    nc.vector.max(m24[:, 16:24], cand[:])
    idxw = TP.tile([128, 24], i32, name="idxw", tag="idxw", bufs=2)
    nc.vector.tensor_scalar(out=idxw[:], in0=m24[:].bitcast(i32),
                            scalar1=mask_lo[:], scalar2=None, op0=AT.bitwise_and)
    nc.vector.tensor_copy(idxf[:, t, :], idxw[:, 0:K])


def _gather_chunk(nc, tc, GP, vTi, gidxi, gc, vmaxTi, vminTi, uTi, acc, l, gch):
    """Gather gch points' K neighbors, reduce max/min/sum/sumsq."""
    f32 = dt.float32
    ni = gch * K
    cs = slice(gc * gch, (gc + 1) * gch)
    gath = GP.tile([128, ni], f32, name="gath", tag="gath")
    nc.gpsimd.ap_gather(out_ap=gath[:], in_ap=vTi[:],
                        idxs_ap=gidxi[:, gc * (ni // 16):(gc + 1) * (ni // 16)],
                        channels=128, num_elems=N, d=1, num_idxs=ni)
    view = gath[:].rearrange("p (n k) -> p n k", k=K)
    nc.vector.tensor_reduce(out=vmaxTi[:, cs], in_=view, axis=AX.X, op=AT.max)
    nc.vector.tensor_reduce(out=vminTi[:, cs], in_=view, axis=AX.X, op=AT.min)
    tmp = GP.tile([128, gch], f32, name="gtmp", tag="gtmp")
    tmp1 = GP.tile([128, 1], f32, name="gtmp1", tag="gtmp1")
    # vsum chunk
    nc.vector.tensor_reduce(out=tmp[:], in_=view, axis=AX.X, op=AT.add)
    nc.vector.tensor_reduce(out=tmp1[:], in_=tmp[:], axis=AX.X, op=AT.add)
    nc.vector.tensor_tensor(out=acc[:, 0:1], in0=acc[:, 0:1], in1=tmp1[:], op=AT.add)
    # suv chunk: sum_n u*vsum
    nc.vector.tensor_tensor(out=tmp[:], in0=tmp[:], in1=uTi[:, cs], op=AT.mult)
    nc.vector.tensor_reduce(out=tmp1[:], in_=tmp[:], axis=AX.X, op=AT.add)
    nc.vector.tensor_tensor(out=acc[:, 1:2], in0=acc[:, 1:2], in1=tmp1[:], op=AT.add)
    # sumsq: square gathered in place, reduce all
    nc.vector.tensor_tensor(out=gath[:], in0=gath[:], in1=gath[:], op=AT.mult)
    nc.vector.tensor_reduce(out=tmp1[:], in_=gath[:], axis=AX.X, op=AT.add)
    nc.vector.tensor_tensor(out=acc[:, 2:3], in0=acc[:, 2:3], in1=tmp1[:], op=AT.add)


def _bn_stats(nc, tc, LP, LD, inp, L, uT, vmaxT, acc, stacked, dbg):
    """Per-channel mean/var over all (b,n,k); returns (s2, t2) [128,1] tiles."""
    f32 = dt.float32
    l, C = L["l"], L["C"]
    cnt = float(B * N * K)
    scr = LP.tile([128, N], f32, name=f"bnscr{l}", tag="scr2")
    st = LP.tile([128, 5], f32, name=f"st{l}")
    if stacked:
        nc.vector.tensor_reduce(out=st[:, 3:4], in_=uT[0][:], axis=AX.X, op=AT.add)
        nc.vector.tensor_tensor(out=scr[:], in0=uT[0][:], in1=uT[0][:], op=AT.mult)
        nc.vector.tensor_reduce(out=st[:, 4:5], in_=scr[:], axis=AX.X, op=AT.add)
        nc.vector.tensor_copy(st[:, 0:3], acc[:, 0:3])
    else:
        nc.vector.tensor_reduce(out=st[:, 3:4], in_=uT[0][:], axis=AX.X, op=AT.add)
        nc.vector.tensor_reduce(out=scr[:, 0:1], in_=uT[1][:], axis=AX.X, op=AT.add)
        nc.vector.tensor_tensor(out=st[:, 3:4], in0=st[:, 3:4], in1=scr[:, 0:1],
                                op=AT.add)
        nc.vector.tensor_tensor(out=scr[:], in0=uT[0][:], in1=uT[0][:], op=AT.mult)
        nc.vector.tensor_reduce(out=st[:, 4:5], in_=scr[:], axis=AX.X, op=AT.add)
        nc.vector.tensor_tensor(out=scr[:], in0=uT[1][:], in1=uT[1][:], op=AT.mult)
        nc.vector.tensor_reduce(out=scr[:, 0:1], in_=scr[:], axis=AX.X, op=AT.add)
        nc.vector.tensor_tensor(out=st[:, 4:5], in0=st[:, 4:5], in1=scr[:, 0:1],
                                op=AT.add)
        nc.vector.tensor_copy(st[:, 0:3], acc[:, 0:3])

    ard_in = LD.tile([128, 5], f32, name=f"arin{l}")
    ard_out = LD.tile([128, 5], f32, name=f"arout{l}", addr_space="Shared")
    nc.sync.dma_start(ard_in[:], st[:])
    nc.gpsimd.collective_compute(
        "AllReduce", AT.add, ins=[ard_in[:]], outs=[ard_out[:]],
        replica_groups=[list(range(N_CORES))])
    ar = LP.tile([128, 5], f32, name=f"ar{l}")
    nc.sync.dma_start(ar[:], ard_out[:])
    if DEBUG and l == 1:
        nc.sync.dma_start(dbg["st1"], ar[:])

    fin = LP.tile([128, 8], f32, name=f"fin{l}")
    if stacked:
        # fold upper half (cloud 1) onto lower: S = ar[0:64] + ar[64:128]
        fold = LP.tile([64, 5], f32, name=f"fold{l}")
        nc.sync.dma_start(fold[:], ar[64:128, :])
        nc.vector.tensor_tensor(out=ar[0:64, :], in0=ar[0:64, :], in1=fold[:],
                                op=AT.add)
        rows = slice(0, 64)
    else:
        rows = slice(0, 128)
    # S1 = K*su + sv ; S2 = K*su2 + 2*suv + sv2
    nc.vector.tensor_scalar(out=fin[rows, 0:1], in0=ar[rows, 3:4], scalar1=float(K),
                            scalar2=None, op0=AT.mult)
    nc.vector.tensor_tensor(out=fin[rows, 0:1], in0=fin[rows, 0:1],
                            in1=ar[rows, 0:1], op=AT.add)
    nc.vector.tensor_scalar(out=fin[rows, 1:2], in0=ar[rows, 4:5], scalar1=float(K),
                            scalar2=None, op0=AT.mult)
    nc.vector.tensor_scalar(out=fin[rows, 2:3], in0=ar[rows, 1:2], scalar1=2.0,
                            scalar2=None, op0=AT.mult)
    nc.vector.tensor_tensor(out=fin[rows, 1:2], in0=fin[rows, 1:2],
                            in1=fin[rows, 2:3], op=AT.add)
    nc.vector.tensor_tensor(out=fin[rows, 1:2], in0=fin[rows, 1:2],
                            in1=ar[rows, 2:3], op=AT.add)
    # m = S1/cnt ; E2 = S2/cnt ; var = E2 - m^2
    nc.vector.tensor_scalar(out=fin[rows, 0:1], in0=fin[rows, 0:1],
                            scalar1=1.0 / cnt, scalar2=None, op0=AT.mult)
    nc.vector.tensor_scalar(out=fin[rows, 1:2], in0=fin[rows, 1:2],
                            scalar1=1.0 / cnt, scalar2=None, op0=AT.mult)
    nc.vector.tensor_tensor(out=fin[rows, 2:3], in0=fin[rows, 0:1],
                            in1=fin[rows, 0:1], op=AT.mult)
    nc.vector.tensor_tensor(out=fin[rows, 1:2], in0=fin[rows, 1:2],
                            in1=fin[rows, 2:3], op=AT.subtract)
    # rstd = 1/sqrt(var+eps)
    nc.vector.tensor_scalar(out=fin[rows, 1:2], in0=fin[rows, 1:2], scalar1=1e-5,
                            scalar2=None, op0=AT.add)
    nc.scalar.activation(fin[rows, 3:4], fin[rows, 1:2], AF.Sqrt)
    nc.vector.reciprocal(fin[rows, 4:5], fin[rows, 3:4])

    gb = LP.tile([128, 2], f32, name=f"gb{l}")
    if stacked:
        for c in range(2):
            nc.sync.dma_start(gb[64 * c:64 * c + 64, 0:1], inp[L["g"]])
            nc.sync.dma_start(gb[64 * c:64 * c + 64, 1:2], inp[L["b"]])
    else:
        nc.sync.dma_start(gb[:, 0:1], inp[L["g"]])
        nc.sync.dma_start(gb[:, 1:2], inp[L["b"]])

    s2 = LP.tile([128, 1], f32, name=f"s2_{l}")
    t2 = LP.tile([128, 1], f32, name=f"t2_{l}")
    nc.vector.tensor_tensor(out=s2[rows], in0=gb[rows, 0:1], in1=fin[rows, 4:5],
                            op=AT.mult)
    nc.vector.tensor_tensor(out=t2[rows], in0=fin[rows, 0:1], in1=s2[rows],
                            op=AT.mult)
    nc.vector.tensor_tensor(out=t2[rows], in0=gb[rows, 1:2], in1=t2[rows],
                            op=AT.subtract)
    if stacked:
        nc.sync.dma_start(s2[64:128, :], s2[0:64, :])
        nc.sync.dma_start(t2[64:128, :], t2[0:64, :])
    return s2, t2


def _apply(nc, tc, LP, L, layers, uT, vmaxT, vminT, s2, t2, stacked,
           xb2, xb3, x3T, halfones, dbg):
    """x_out = max(lrelu(s*(u+vmax)+t), lrelu(s*(u+vmin)+t)); route to next xb."""
    f32 = dt.float32
    l = L["l"]
    nV = 1 if stacked else BL
    for i in range(nV):
        z1 = LP.tile([128, N], f32, name="z1", tag="scr1")
        z2 = LP.tile([128, N], f32, name="z2", tag="scr2")
        nc.vector.tensor_tensor(out=z1[:], in0=uT[i][:], in1=vmaxT[i][:], op=AT.add)
        nc.vector.tensor_tensor(out=z2[:], in0=uT[i][:], in1=vminT[i][:], op=AT.add)
        # y = s*z + t on ACT; lrelu + max on DVE
        nc.scalar.activation(z1[:], z1[:], AF.Identity, bias=t2[:], scale=s2[:])
        nc.scalar.activation(z2[:], z2[:], AF.Identity, bias=t2[:], scale=s2[:])
        nc.vector.scalar_tensor_tensor(out=z1[:], in0=z1[:], scalar=0.2, in1=z1[:],
                                       op0=AT.mult, op1=AT.max)
        nc.vector.scalar_tensor_tensor(out=z2[:], in0=z2[:], scalar=0.2, in1=z2[:],
                                       op0=AT.mult, op1=AT.max)
        xout = x3T[i] if l == 3 else z1
        nc.vector.tensor_tensor(out=xout[:], in0=z1[:], in1=z2[:], op=AT.max)
        if l < 3:
            nxt = xb2 if l == 1 else xb3
            # cloud 0 rows 0:64 stay put; cloud 1 rows 64:128 shift via DMA
            nc.sync.dma_start(nxt[0][0:64, :], xout[0:64, :])
            nc.sync.dma_start(nxt[1][0:64, :], xout[64:128, :])
            if DEBUG:
                nc.sync.dma_start(dbg[f"x{l}"], xout[:])
        elif DEBUG:
            nc.sync.dma_start(dbg["x3"][i], xout[:])


def _final_conv(nc, tc, inp, xb2, xb3, x3T, FP, FD):
    """xf^T = Wf^T xc^T per cloud; per-channel max/min over n + BN sums + AR.
    Returns the all-gathered gfeat DRAM tile [16, 1024]."""
    f32 = dt.float32

    wf0 = FP.tile([64, 1024], f32)
    wf1 = FP.tile([64, 1024], f32)
    wf2 = FP.tile([128, 1024], f32)
    nc.sync.dma_start(wf0[:], inp["Wf"][0:64, :])
    nc.sync.dma_start(wf1[:], inp["Wf"][64:128, :])
    nc.sync.dma_start(wf2[:], inp["Wf"][128:256, :])

    ssum = FP.tile([128, 8], f32)
    ssum2 = FP.tile([128, 8], f32)
    smax = FP.tile([128, 16], f32)
    smin = FP.tile([128, 16], f32)
    nc.vector.memset(ssum[:], 0.0)
    nc.vector.memset(ssum2[:], 0.0)
    nc.vector.memset(smax[:], NEG_BIG)
    nc.vector.memset(smin[:], -NEG_BIG)

    f32r = dt.float32r
    with tc.tile_pool(name="pfc", bufs=3, space="PSUM") as PFC, \
         tc.tile_pool(name="fctmp", bufs=2) as FT:
        for c in range(BL):
            for ot in range(8):
                ws = slice(ot * 128, (ot + 1) * 128)
                for nch in range(4):
                    cs = slice(nch * 512, (nch + 1) * 512)
                    pb = PFC.tile([128, 512], f32, tag="pfc")
                    nc.tensor.matmul(pb[:], wf0[:, ws], xb2[c][0:64, cs],
                                     start=True, stop=False)
                    nc.tensor.matmul(pb[:], wf1[:, ws], xb3[c][0:64, cs],
                                     start=False, stop=False)
                    nc.tensor.matmul(pb[:], wf2[:, ws], x3T[c][:, cs],
                                     start=False, stop=True)
                    t1 = FT.tile([128, 1], f32, tag="fct1")
                    sq = FT.tile([128, 512], f32, tag="fcsq")
                    mcol = smax[:, c * 8 + ot:c * 8 + ot + 1]
                    ncol = smin[:, c * 8 + ot:c * 8 + ot + 1]
                    nc.vector.tensor_reduce(out=t1[:], in_=pb[:], axis=AX.X, op=AT.max)
                    nc.vector.tensor_tensor(out=mcol, in0=mcol, in1=t1[:], op=AT.max)
                    nc.vector.tensor_reduce(out=t1[:], in_=pb[:], axis=AX.X, op=AT.min)
                    nc.vector.tensor_tensor(out=ncol, in0=ncol, in1=t1[:], op=AT.min)
                    scol = ssum[:, ot:ot + 1]
                    qcol = ssum2[:, ot:ot + 1]
                    nc.vector.tensor_reduce(out=t1[:], in_=pb[:], axis=AX.X, op=AT.add)
                    nc.vector.tensor_tensor(out=scol, in0=scol, in1=t1[:], op=AT.add)
                    nc.scalar.activation(sq[:], pb[:], AF.Square)
                    nc.vector.tensor_reduce(out=t1[:], in_=sq[:], axis=AX.X, op=AT.add)
                    nc.vector.tensor_tensor(out=qcol, in0=qcol, in1=t1[:], op=AT.add)

    ard_in = FD.tile([128, 16], f32)
    ard_out = FD.tile([128, 16], f32, addr_space="Shared")
    nc.sync.dma_start(ard_in[:, 0:8], ssum[:])
    nc.sync.dma_start(ard_in[:, 8:16], ssum2[:])
    nc.gpsimd.collective_compute(
        "AllReduce", AT.add, ins=[ard_in[:]], outs=[ard_out[:]],
        replica_groups=[list(range(N_CORES))])
    ar = FP.tile([128, 16], f32)
    nc.sync.dma_start(ar[:], ard_out[:])

    cntf = float(B * N)
    m = FP.tile([128, 8], f32)
    var = FP.tile([128, 8], f32)
    nc.vector.tensor_scalar(out=m[:], in0=ar[:, 0:8], scalar1=1.0 / cntf,
                            scalar2=None, op0=AT.mult)
    nc.vector.tensor_scalar(out=var[:], in0=ar[:, 8:16], scalar1=1.0 / cntf,
                            scalar2=None, op0=AT.mult)
    sq2 = FP.tile([128, 8], f32)
    nc.vector.tensor_tensor(out=sq2[:], in0=m[:], in1=m[:], op=AT.mult)
    nc.vector.tensor_tensor(out=var[:], in0=var[:], in1=sq2[:], op=AT.subtract)
    nc.vector.tensor_scalar(out=var[:], in0=var[:], scalar1=1e-5, scalar2=None,
                            op0=AT.add)
    nc.scalar.activation(var[:], var[:], AF.Sqrt)
    rstd = FP.tile([128, 8], f32)
    nc.vector.reciprocal(rstd[:], var[:])
    gfb = FP.tile([128, 16], f32)
    nc.sync.dma_start(gfb[:, 0:8], inp["gf"].rearrange("(a p) -> p a", p=128))
    nc.sync.dma_start(gfb[:, 8:16], inp["bf"].rearrange("(a p) -> p a", p=128))
    sF = FP.tile([128, 8], f32)
    tF = FP.tile([128, 8], f32)
    nc.vector.tensor_tensor(out=sF[:], in0=gfb[:, 0:8], in1=rstd[:], op=AT.mult)
    nc.vector.tensor_tensor(out=tF[:], in0=m[:], in1=sF[:], op=AT.mult)
    nc.vector.tensor_tensor(out=tF[:], in0=gfb[:, 8:16], in1=tF[:], op=AT.subtract)

    # gfeat[cl, o] = max(lrelu(s*smax+t), lrelu(s*smin+t))
    gfe = FP.tile([128, 16], f32)   # cols cl*8+ot
    y1 = FP.tile([128, 1], f32)
    y2 = FP.tile([128, 1], f32)
    for c in range(BL):
        for ot in range(8):
            col = c * 8 + ot
            nc.vector.tensor_scalar(out=y1[:], in0=smax[:, col:col + 1],
                                    scalar1=sF[:, ot:ot + 1], scalar2=tF[:, ot:ot + 1],
                                    op0=AT.mult, op1=AT.add)
            nc.vector.scalar_tensor_tensor(out=y1[:], in0=y1[:], scalar=0.2,
                                           in1=y1[:], op0=AT.mult, op1=AT.max)
            nc.vector.tensor_scalar(out=y2[:], in0=smin[:, col:col + 1],
                                    scalar1=sF[:, ot:ot + 1], scalar2=tF[:, ot:ot + 1],
                                    op0=AT.mult, op1=AT.add)
            nc.vector.scalar_tensor_tensor(out=y2[:], in0=y2[:], scalar=0.2,
                                           in1=y2[:], op0=AT.mult, op1=AT.max)
            nc.vector.tensor_tensor(out=gfe[:, col:col + 1], in0=y1[:], in1=y2[:],
                                    op=AT.max)

    agi = FD.tile([BL, 1024], f32)
    ago = FD.tile([B, 1024], f32, addr_space="Shared")
    nc.sync.dma_start(
        agi[:].rearrange("c (ot p) -> p c ot", p=128), gfe[:])
    nc.gpsimd.collective_compute(
        "AllGather", AT.bypass, ins=[agi[:]], outs=[ago[:]],
        replica_groups=[list(range(N_CORES))])
    return ago


def _classifier(nc, tc, inp, ago, out_ap, dbg, CP):
    """Replicated tiny classifier on the all-gathered gfeat [16, 1024]."""
    f32 = dt.float32

    gT = CP.tile([128, 8 * B], f32)   # [p, (ot, b)]
    for ot in range(8):
        nc.gpsimd.dma_start(
            gT[:, ot * B:(ot + 1) * B],
            ago[:, ot * 128:(ot + 1) * 128].rearrange("b p -> p b"))
    if DEBUG:
        nc.sync.dma_start(dbg["gf"], ago[:])

    w4 = CP.tile([128, 8 * 512], f32)
    nc.sync.dma_start(w4[:].rearrange("p (rc o) -> p rc o", rc=8),
                      inp["W4"].rearrange("(rc p) o -> p rc o", p=128))
    w5 = CP.tile([128, 4 * 256], f32)
    nc.sync.dma_start(w5[:].rearrange("p (rc o) -> p rc o", rc=4),
                      inp["W5"].rearrange("(rc p) o -> p rc o", p=128))
    w6 = CP.tile([128, 2 * NCLS], f32)
    nc.sync.dma_start(w6[:].rearrange("p (rc o) -> p rc o", rc=2),
                      inp["W6"].rearrange("(rc p) o -> p rc o", p=128))

    def bn_block(pre, nblk, gname, bname):
        """pre: psum [128, nblk*B]; returns SBUF tile post BN+LReLU."""
        h = CP.tile([128, nblk * B], f32, name=f"h_{gname}")
        gb = CP.tile([128, 2 * nblk], f32, name=f"gb_{gname}")
        nc.sync.dma_start(gb[:, 0:nblk],
                          inp[gname].rearrange("(a p) -> p a", p=128))
        nc.sync.dma_start(gb[:, nblk:2 * nblk],
                          inp[bname].rearrange("(a p) -> p a", p=128))
        mt = CP.tile([128, 1], f32, name=f"mt_{gname}")
        vt = CP.tile([128, 1], f32, name=f"vt_{gname}")
        sq = CP.tile([128, B], f32, name=f"sqc_{gname}", tag=f"sq_{gname}")
        for j in range(nblk):
            bs = slice(j * B, (j + 1) * B)
            nc.vector.tensor_reduce(out=mt[:], in_=pre[:, bs], axis=AX.X, op=AT.add)
            nc.vector.tensor_scalar(out=mt[:], in0=mt[:], scalar1=1.0 / B,
                                    scalar2=None, op0=AT.mult)
            nc.scalar.activation(sq[:], pre[:, bs], AF.Square)
            nc.vector.tensor_reduce(out=vt[:], in_=sq[:], axis=AX.X, op=AT.add)
            nc.vector.tensor_scalar(out=vt[:], in0=vt[:], scalar1=1.0 / B,
                                    scalar2=None, op0=AT.mult)
            nc.vector.tensor_tensor(out=sq[:, 0:1], in0=mt[:], in1=mt[:], op=AT.mult)
            nc.vector.tensor_tensor(out=vt[:], in0=vt[:], in1=sq[:, 0:1],
                                    op=AT.subtract)
            nc.vector.tensor_scalar(out=vt[:], in0=vt[:], scalar1=1e-5,
                                    scalar2=None, op0=AT.add)
            nc.scalar.activation(vt[:], vt[:], AF.Sqrt)
            nc.vector.reciprocal(vt[:], vt[:])
            nc.vector.tensor_tensor(out=vt[:], in0=vt[:], in1=gb[:, j:j + 1],
                                    op=AT.mult)   # s
            nc.vector.tensor_tensor(out=mt[:], in0=mt[:], in1=vt[:], op=AT.mult)
            nc.vector.tensor_tensor(out=mt[:], in0=gb[:, nblk + j:nblk + j + 1],
                                    in1=mt[:], op=AT.subtract)   # t
            nc.vector.tensor_scalar(out=h[:, bs], in0=pre[:, bs], scalar1=vt[:],
                                    scalar2=mt[:], op0=AT.mult, op1=AT.add)
            nc.vector.scalar_tensor_tensor(out=h[:, bs], in0=h[:, bs], scalar=0.2,
                                           in1=h[:, bs], op0=AT.mult, op1=AT.max)
        return h

    with tc.tile_pool(name="pcls", bufs=2, space="PSUM") as PC:
        p1 = PC.tile([128, 4 * B], f32)
        for o2 in range(4):
            for rc in range(8):
                nc.tensor.matmul(p1[:, o2 * B:(o2 + 1) * B],
                                 w4[:, rc * 512 + o2 * 128:rc * 512 + (o2 + 1) * 128],
                                 gT[:, rc * B:(rc + 1) * B],
                                 start=(rc == 0), stop=(rc == 7))
        h1 = bn_block(p1[:], 4, "g4", "be4")

        p2 = PC.tile([128, 2 * B], f32)
        for o3 in range(2):
            for rc in range(4):
                nc.tensor.matmul(p2[:, o3 * B:(o3 + 1) * B],
                                 w5[:, rc * 256 + o3 * 128:rc * 256 + (o3 + 1) * 128],
                                 h1[:, rc * B:(rc + 1) * B],
                                 start=(rc == 0), stop=(rc == 3))
        h2 = bn_block(p2[:], 2, "g5", "be5")

        p3 = PC.tile([NCLS, B], f32)
        for rc in range(2):
            nc.tensor.matmul(p3[:], w6[:, rc * NCLS:(rc + 1) * NCLS],
                             h2[:, rc * B:(rc + 1) * B],
                             start=(rc == 0), stop=(rc == 1))
        b6 = CP.tile([NCLS, 1], f32)
        nc.sync.dma_start(b6[:], inp["b6"])
        lg = CP.tile([NCLS, B], f32)
        nc.vector.tensor_scalar(out=lg[:], in0=p3[:], scalar1=b6[:], scalar2=None,
                                op0=AT.add)
        nc.sync.dma_start(out_ap.rearrange("b c -> c b"), lg[:])


def _prep_inputs(inputs):
    """Host-side: per-core shards + weight splits (layout-only prep)."""
    pos = np.ascontiguousarray(inputs["pos"], dtype=np.float32)
    W1, W2, W3 = inputs["W1"], inputs["W2"], inputs["W3"]
    common = {
        "wu1": np.ascontiguousarray(W1[0:3] - W1[3:6], np.float32),
        "wv1": np.ascontiguousarray(W1[3:6], np.float32),
        "wu2": np.ascontiguousarray(W2[0:64] - W2[64:128], np.float32),
        "wv2": np.ascontiguousarray(W2[64:128], np.float32),
        "wu3": np.ascontiguousarray(W3[0:64] - W3[64:128], np.float32),
        "wv3": np.ascontiguousarray(W3[64:128], np.float32),
    }
    for k in ("Wf", "gf", "bf", "W4", "g4", "be4", "W5", "g5", "be5", "W6"):
        common[k] = np.ascontiguousarray(inputs[k], np.float32)
    for k in ("g1", "b1", "g2", "b2", "g3", "b3", "b6"):
        common[k] = np.ascontiguousarray(inputs[k], np.float32).reshape(-1, 1)
    in_maps = []
    for i in range(N_CORES):
        m = dict(common)
        m["posT"] = np.ascontiguousarray(
            pos[i * BL:(i + 1) * BL].transpose(0, 2, 1), np.float32)
        in_maps.append(m)
    return in_maps


def kernel(**inputs) -> np.ndarray:
    if "nc" not in _CACHE:
        _CACHE["nc"] = _build()
    nc = _CACHE["nc"]
    in_maps = _prep_inputs(inputs)
    res = bass_utils.run_bass_kernel_spmd(
        nc, in_maps, core_ids=list(range(N_CORES)),
        trace=bool(int(os.environ.get("DGCNN_TRACE", "0"))))
    _CACHE["last_results"] = res
    return np.ascontiguousarray(res.results[0]["out"])
